# revision 11
# baseline (speedup 1.0000x reference)
"""CurricularFace loss kernel for Trainium2, classification-parallel over 8 cores.

Contract: kernel(**inputs) takes the FULL inputs (embeddings [512,512] f32,
kernel [512,100000] f32, label [512] int, t [1] f32) and returns the FULL
[512,100000] f32 output.

Strategy (partial-FC style):
  - kernel (the class weight matrix) is column-sharded 8 x 12500.
  - embeddings^T, the 512 gathered label columns kernel[:, label], and t are
    replicated; every core redundantly computes all 512 target logits and the
    t EMA from the tiny label-column matrix, so no collectives are needed.
  - Per core: cosine = norm(emb) @ norm(kshard) via fp16 matmuls (fp32 PSUM
    accum); ScalarE computes both branch values directly from PSUM
    (U = S*cosine, Q = S*(cosine + t_new/2)^2, folding the embedding row-norm
    into the per-partition activation scale); VectorE computes the
    hard-negative mask and blends with copy_predicated.
  - The per-row target column is overwritten on the host with the
    device-computed S*final_target values (pure data movement).
"""

import math
from contextlib import ExitStack

import numpy as np

import concourse.bacc as bacc
import concourse.tile as tile
from concourse import mybir
from concourse.alu_op_type import AluOpType
from concourse.bass_utils import run_bass_kernel_spmd

S = 30.0
M = 0.5
COS_M = math.cos(M)
SIN_M = math.sin(M)
THRESHOLD = math.cos(math.pi - M)
MM = math.sin(math.pi - M) * M
SQRT_S = math.sqrt(S)

B, D, C = 512, 512, 100000
NCORES = 8
CS = C // NCORES  # columns per core
P = 128
KC = D // P  # contraction chunks
MCH = B // P  # output row chunks
NT = 500  # matmul free-dim tile (one PSUM bank of fp32)

F32 = mybir.dt.float32
F16 = mybir.dt.float16

_BUILT = {}
last_results = None


def _build(cs):
    """Build the single-core Bass program (same program runs SPMD on 8 cores)."""
    ntiles = cs // NT
    nc = bacc.Bacc("TRN2", target_bir_lowering=False, debug=False, num_devices=NCORES)

    embT = nc.dram_tensor("embT", [D, B], F32, kind="ExternalInput").ap()
    klab = nc.dram_tensor("klab", [D, B], F32, kind="ExternalInput").ap()
    ksh = nc.dram_tensor("ksh", [D, cs], F32, kind="ExternalInput").ap()
    t_in = nc.dram_tensor("t", [1, 1], F32, kind="ExternalInput").ap()
    out = nc.dram_tensor("out", [B, cs], F32, kind="ExternalOutput").ap()
    ft_out = nc.dram_tensor("ft", [1, B], F32, kind="ExternalOutput").ap()

    Act = mybir.ActivationFunctionType
    X = mybir.AxisListType.X

    with tile.TileContext(nc) as tc:
        with (
            tc.tile_pool(name="singles", bufs=1) as singles,
            tc.tile_pool(name="vecs", bufs=1) as vecs,
            tc.tile_pool(name="dram", bufs=1, space="DRAM") as dpool,
        ):
            _setup_stack = ExitStack()
            setup = _setup_stack.enter_context(tc.tile_pool(name="setup", bufs=3))
            spsum = _setup_stack.enter_context(
                tc.tile_pool(name="spsum", bufs=1, space="PSUM")
            )
            # ---------------- setup: norms, target logits, t EMA ------------
            ones = singles.tile([P, 1], F32, tag="ones")
            nc.vector.memset(ones, 1.0)
            ones_row = singles.tile([1, P], F32, tag="ones_row")
            nc.vector.memset(ones_row, 1.0)

            e16 = []  # persistent fp16 embT chunks [128, 512]
            ps_e = spsum.tile([1, B], F32, tag="ps_e")
            ps_l = spsum.tile([1, B], F32, tag="ps_l")
            ps_tl = spsum.tile([1, B], F32, tag="ps_tl")
            for k in range(KC):
                ksl = slice(k * P, (k + 1) * P)
                ech = setup.tile([P, B], F32, tag="ech")
                nc.sync.dma_start(out=ech, in_=embT[ksl, :])
                e16k = singles.tile([P, B], F16, tag=f"e16_{k}")
                nc.vector.tensor_copy(e16k, ech)
                e16.append(e16k)

                lch = setup.tile([P, B], F32, tag="lch")
                nc.sync.dma_start(out=lch, in_=klab[ksl, :])

                esq = setup.tile([P, B], F32, tag="esq")
                nc.scalar.activation(esq, ech, Act.Square)
                lsq = setup.tile([P, B], F32, tag="lsq")
                nc.scalar.activation(lsq, lch, Act.Square)
                prod = setup.tile([P, B], F32, tag="prod")
                nc.vector.tensor_mul(prod, ech, lch)

                st, sp = (k == 0), (k == KC - 1)
                nc.tensor.matmul(ps_e, ones, esq, start=st, stop=sp)
                nc.tensor.matmul(ps_l, ones, lsq, start=st, stop=sp)
                nc.tensor.matmul(ps_tl, ones, prod, start=st, stop=sp)

            def rsqrt_newton(ssq_psum, tag):
                # r = 1/sqrt(ssq) with one Newton step (ACT Rsqrt is banned).
                ssq = vecs.tile([1, B], F32, tag=f"{tag}_ssq")
                nc.vector.tensor_copy(ssq, ssq_psum)
                rec = vecs.tile([1, B], F32, tag=f"{tag}_rec")
                nc.vector.reciprocal(rec, ssq)
                r0 = vecs.tile([1, B], F32, tag=f"{tag}_r0")
                nc.scalar.activation(r0, rec, Act.Sqrt)
                r2 = vecs.tile([1, B], F32, tag=f"{tag}_r2")
                nc.scalar.activation(r2, r0, Act.Square)
                p = vecs.tile([1, B], F32, tag=f"{tag}_p")
                nc.vector.tensor_mul(p, r2, ssq)
                q = vecs.tile([1, B], F32, tag=f"{tag}_q")
                nc.vector.tensor_scalar(q, p, -0.5, 1.5, AluOpType.mult, AluOpType.add)
                r1 = vecs.tile([1, B], F32, tag=f"{tag}_r1")
                nc.vector.tensor_mul(r1, r0, q)
                return r1

            rne = rsqrt_newton(ps_e, "e")  # 1/||emb_b||
            rnl = rsqrt_newton(ps_l, "l")  # 1/||kernel[:,label_b]||

            tl = vecs.tile([1, B], F32, tag="tl")  # target logits
            nc.vector.tensor_copy(tl, ps_tl)
            nc.vector.tensor_mul(tl, tl, rne)
            nc.vector.tensor_mul(tl, tl, rnl)
            nc.vector.tensor_scalar(tl, tl, 1.0, -1.0, AluOpType.min, AluOpType.max)

            # t_new = 0.99*t + 0.01*mean(tl)
            ssum = vecs.tile([1, 1], F32, tag="ssum")
            nc.vector.reduce_sum(ssum, tl, axis=X)
            tsb = vecs.tile([1, 1], F32, tag="tsb")
            nc.sync.dma_start(out=tsb, in_=t_in)
            tnew = vecs.tile([1, 1], F32, tag="tnew")
            nc.vector.tensor_scalar_mul(tnew, tsb, 0.99)
            tpart = vecs.tile([1, 1], F32, tag="tpart")
            nc.vector.tensor_scalar_mul(tpart, ssum, 0.01 / B)
            nc.vector.tensor_add(tnew, tnew, tpart)

            # sin_theta = sqrt(1 - tl^2), Newton-refined
            s2n = vecs.tile([1, B], F32, tag="s2n")
            nc.scalar.activation(s2n, tl, Act.Square)
            nc.vector.tensor_scalar(s2n, s2n, -1.0, 1.0, AluOpType.mult, AluOpType.add)
            st_ = vecs.tile([1, B], F32, tag="st")
            nc.scalar.activation(st_, s2n, Act.Sqrt)
            rz = vecs.tile([1, B], F32, tag="rz")
            nc.vector.reciprocal(rz, st_)
            w_ = vecs.tile([1, B], F32, tag="w")
            nc.vector.tensor_mul(w_, s2n, rz)
            nc.vector.tensor_add(st_, st_, w_)
            nc.vector.tensor_scalar_mul(st_, st_, 0.5)

            # cos(theta+m) = tl*COS_M - sin_theta*SIN_M
            ctm = vecs.tile([1, B], F32, tag="ctm")
            nc.vector.tensor_scalar_mul(ctm, st_, -SIN_M)
            tlc = vecs.tile([1, B], F32, tag="tlc")
            nc.vector.tensor_scalar_mul(tlc, tl, COS_M)
            nc.vector.tensor_add(ctm, ctm, tlc)

            # final_target = where(tl > THRESHOLD, ctm, tl - MM), scaled by S
            ftv = vecs.tile([1, B], F32, tag="ftv")
            nc.vector.tensor_scalar_add(ftv, tl, -MM)
            m2 = vecs.tile([1, B], mybir.dt.uint8, tag="m2")
            nc.vector.tensor_scalar(m2, tl, THRESHOLD, None, AluOpType.is_gt)
            nc.vector.copy_predicated(ftv, m2, ctm)
            nc.vector.tensor_scalar_mul(ftv, ftv, S)
            nc.sync.dma_start(out=ft_out, in_=ftv)

            # per-row activation params, rearranged to [128, MCH] via DRAM
            auv = vecs.tile([1, B], F32, tag="auv")
            nc.vector.tensor_scalar_mul(auv, rne, S)
            aqv = vecs.tile([1, B], F32, tag="aqv")
            nc.vector.tensor_scalar_mul(aqv, rne, SQRT_S)
            cthv = vecs.tile([1, B], F32, tag="cthv")
            nc.vector.tensor_scalar_mul(cthv, ctm, S)
            bqv = vecs.tile([1, 1], F32, tag="bqv")
            nc.vector.tensor_scalar_mul(bqv, tnew, SQRT_S * 0.5)

            scratch = dpool.tile([4, B], F32)
            nc.sync.dma_start(out=scratch[0:1, :], in_=auv)
            nc.sync.dma_start(out=scratch[1:2, :], in_=aqv)
            nc.sync.dma_start(out=scratch[2:3, :], in_=cthv)
            nc.sync.dma_start(out=scratch[3:4, 0:1], in_=bqv)

            au = singles.tile([P, MCH], F32, tag="au")
            aq = singles.tile([P, MCH], F32, tag="aq")
            cth = singles.tile([P, MCH], F32, tag="cth")
            bias_q = singles.tile([P, 1], F32, tag="bias_q")
            nc.sync.dma_start(out=au, in_=scratch[0, :].rearrange("(c p) -> p c", p=P))
            nc.sync.dma_start(out=aq, in_=scratch[1, :].rearrange("(c p) -> p c", p=P))
            nc.sync.dma_start(out=cth, in_=scratch[2, :].rearrange("(c p) -> p c", p=P))
            nc.sync.dma_start(out=bias_q, in_=scratch[3:4, 0:1].to_broadcast([P, 1]))

            _setup_stack.close()

            # ---------------- main loop: 25 column tiles of 500 -------------
            with (
                tc.tile_pool(name="kr", bufs=3) as krp,
                tc.tile_pool(name="k16", bufs=3) as k16p,
                tc.tile_pool(name="sqp", bufs=3) as sqp,
                tc.tile_pool(name="cn", bufs=3) as cnp,
                tc.tile_pool(name="uo", bufs=6) as uop,
                tc.tile_pool(name="qq", bufs=6) as qqp,
                tc.tile_pool(name="mk", bufs=6) as mkp,
                tc.tile_pool(name="mm", bufs=4, space="PSUM") as mmp,
                tc.tile_pool(name="cnps", bufs=2, space="PSUM") as cnpsp,
                tc.tile_pool(name="ssps", bufs=2, space="PSUM") as sspsp,
            ):
                for n in range(ntiles):
                    nsl = slice(n * NT, (n + 1) * NT)
                    kr = krp.tile([P, KC, NT], F32, tag="kr")
                    for k in range(KC):
                        nc.sync.dma_start(
                            out=kr[:, k, :], in_=ksh[k * P : (k + 1) * P, nsl]
                        )
                    # column sum-squares of this tile via Square + ones-matmul
                    sq = sqp.tile([P, KC, NT], F32, tag="sq")
                    ssq_ps = sspsp.tile([1, NT], F32, tag="ssq_ps")
                    for k in range(KC):
                        nc.scalar.activation(sq[:, k, :], kr[:, k, :], Act.Square)
                        nc.tensor.matmul(
                            ssq_ps,
                            ones,
                            sq[:, k, :],
                            start=(k == 0),
                            stop=(k == KC - 1),
                        )
                    # cnr = 1/sqrt(ssq), one Newton step
                    cssq = cnp.tile([1, NT], F32, tag="cssq")
                    nc.vector.tensor_copy(cssq, ssq_ps)
                    crec = cnp.tile([1, NT], F32, tag="crec")
                    nc.vector.reciprocal(crec, cssq)
                    cr0 = cnp.tile([1, NT], F32, tag="cr0")
                    nc.scalar.activation(cr0, crec, Act.Sqrt)
                    cr2 = cnp.tile([1, NT], F32, tag="cr2")
                    nc.scalar.activation(cr2, cr0, Act.Square)
                    cp_ = cnp.tile([1, NT], F32, tag="cp_")
                    nc.vector.tensor_mul(cp_, cr2, cssq)
                    cq = cnp.tile([1, NT], F32, tag="cq")
                    nc.vector.tensor_scalar(
                        cq, cp_, -0.5, 1.5, AluOpType.mult, AluOpType.add
                    )
                    cnr = cnp.tile([1, NT], F32, tag="cnr")
                    nc.vector.tensor_mul(cnr, cr0, cq)
                    # broadcast cnr across partitions via K=1 matmul into PSUM
                    cnr_bc = cnpsp.tile([P, NT], F32, tag="cnr_bc")
                    nc.tensor.matmul(cnr_bc, ones_row, cnr, start=True, stop=True)
                    # fused column-normalize + fp16 cast
                    k16 = k16p.tile([P, KC, NT], F16, tag="k16")
                    for k in range(KC):
                        nc.vector.tensor_mul(k16[:, k, :], kr[:, k, :], cnr_bc)
                    for m in range(MCH):
                        msl = slice(m * P, (m + 1) * P)
                        ps = mmp.tile([P, NT], F32, tag="ps")
                        for k in range(KC):
                            nc.tensor.matmul(
                                ps,
                                e16[k][:, msl],
                                k16[:, k, :],
                                start=(k == 0),
                                stop=(k == KC - 1),
                            )
                        u = uop.tile([P, NT], F32, tag="u")
                        nc.scalar.activation(
                            u, ps, Act.Copy, bias=0.0, scale=au[:, m : m + 1]
                        )
                        q = qqp.tile([P, NT], F32, tag="q")
                        nc.scalar.activation(
                            q, ps, Act.Square, bias=bias_q, scale=aq[:, m : m + 1]
                        )
                        msk = mkp.tile([P, NT], mybir.dt.uint8, tag="msk")
                        nc.vector.tensor_scalar(
                            msk, u, cth[:, m : m + 1], None, AluOpType.is_gt
                        )
                        nc.vector.copy_predicated(u, msk, q)
                        nc.sync.dma_start(out=out[msl, nsl], in_=u)
    nc.compile()
    return nc


def _get_nc(cs=CS):
    if cs not in _BUILT:
        _BUILT[cs] = _build(cs)
    return _BUILT[cs]


def kernel(embeddings, kernel, label, t):
    embeddings = np.ascontiguousarray(np.asarray(embeddings, dtype=np.float32))
    kmat = np.asarray(kernel, dtype=np.float32)
    label_i = np.asarray(label).astype(np.int64)
    t_np = np.asarray(t, dtype=np.float32).reshape(1, 1)

    embT = np.ascontiguousarray(embeddings.T)
    klab = np.ascontiguousarray(kmat[:, label_i])

    nc = _get_nc(CS)
    in_maps = []
    for i in range(NCORES):
        in_maps.append(
            {
                "embT": embT,
                "klab": klab,
                "ksh": np.ascontiguousarray(kmat[:, i * CS : (i + 1) * CS]),
                "t": t_np,
            }
        )
    global last_results
    last_results = run_bass_kernel_spmd(nc, in_maps, list(range(NCORES)))
    res = last_results.results

    full = np.concatenate([res[i]["out"] for i in range(NCORES)], axis=1)
    ft = res[0]["ft"].reshape(B)
    full[np.arange(B), label_i] = ft
    return full
